# revision 36
# baseline (speedup 1.0000x reference)
"""Multi-head attention forward on 8 Trainium2 NeuronCores.

Problem (all shapes hardcoded): B=2, S=2048, D=1024, H=16, HD=64
    q = relu(x @ Wq + bq); k = relu(x @ Wk + bk); v = relu(x @ Wv + bv)
    attn = softmax(q k^T / sqrt(HD)) per (batch, head)
    out = relu((attn @ v) @ Wo + bo)

Sharding: head-parallel for QKV+attention (2 heads per core, both batches).
Token ownership for the output projection is sliver-interleaved: core j owns
tokens [qc*512 + j*64, qc*512 + (j+1)*64) for every query chunk qc.  Context
ships through AllToAlls sized so the CC stream never backlogs: batch-0 and
batch-1's first half ship per half-batch ([8,128,128]); batch-1's qc2/qc3
ship per-chunk so only a 64KB exchange sits in the tail.

Device schedule (per core):
  - Q^T/K^T as stacked 2-head [128, T] tiles; the two per-head score matmuls
    (K_c=64) land on different PE row groups and stream concurrently.
  - scores transposed S^T[k, q]; exp on ACT from PSUM with scale=1/8 and
    bias=-ln(32) (keeps weights in e4m3 range; cancels in the softmax).
    exp output is fp8 packed in kb pairs; attention*V runs as fp8 DoubleRow
    matmuls contracting 256 keys per instruction.  V_aug rows 64:128
    replicate the softmax denominator (ones columns).
  - normalize: regular-copy partition shift + reciprocal_approx_fast + mult.
  - all projections/gathers/output-projection blocks ride as small (~1.3us)
    filler units at deadline-driven positions inside the attention loops --
    PE executes its queue in order, so any burst longer than ~2us starves
    ACT (the bottleneck engine is whichever exceeds ~1.1us/iteration).
"""

import os
import sys

import numpy as np

for _p in ("/opt/trn_rl_repo",):
    if os.path.isdir(_p) and _p not in sys.path:
        sys.path.append(_p)

import ml_dtypes

B, S, D, H = 2, 2048, 1024, 16
HD = D // H          # 64
NCORES = 8
T = B * S            # 4096 flattened tokens
DC = D // NCORES     # 128 head-dim columns per core (2 heads)
P = 128
KT_TILES = D // P    # 8 contraction tiles over d_model
SB_Q = S // 512      # 4 query chunks per batch
KB = S // P          # 16 key blocks per batch
NTB = T // P         # 32 token blocks
SLIV = 512 // NCORES  # 64-token sliver per (qc, dest core)
CH = SB_Q * SLIV     # 256 tokens per core per batch

_bf = ml_dtypes.bfloat16

PROFILE = False
PROFILE_CORES = [0]
LAST_RESULTS = None
USE_FAST_RECIP = True

_CACHE = {}


def _build(with_bias_v, with_bias_o, with_bias_qk):
    import concourse.mybir as mybir
    import concourse.tile as tile
    from concourse import bacc
    from concourse.bass import ds, ts
    from contextlib import ExitStack

    f32 = mybir.dt.float32
    bf16 = mybir.dt.bfloat16
    DT = bf16
    AF = mybir.ActivationFunctionType

    nc = bacc.Bacc("TRN2", target_bir_lowering=False, debug=False,
                   num_devices=NCORES)

    xT = nc.dram_tensor("xT", [D, T], DT, kind="ExternalInput")
    wq = nc.dram_tensor("wq", [D, DC], DT, kind="ExternalInput")
    wk = nc.dram_tensor("wk", [D, DC], DT, kind="ExternalInput")
    wv = nc.dram_tensor("wv", [D, DC], DT, kind="ExternalInput")
    wo = nc.dram_tensor("wo", [D, D], DT, kind="ExternalInput")
    bqd = nc.dram_tensor("bqv", [DC, 1], f32, kind="ExternalInput")
    bkd = nc.dram_tensor("bkv", [DC, 1], f32, kind="ExternalInput")
    bvd = nc.dram_tensor("bvv", [1, DC], DT, kind="ExternalInput")
    bod = nc.dram_tensor("bov", [1, D], DT, kind="ExternalInput")
    out = nc.dram_tensor("out", [B * CH, D], f32, kind="ExternalOutput")

    with tile.TileContext(nc) as tc, ExitStack() as ctx:
        sb = ctx.enter_context(tc.tile_pool(name="persist", bufs=1))
        dram = ctx.enter_context(tc.tile_pool(name="dram", bufs=1, space="DRAM"))
        psum = ctx.enter_context(tc.tile_pool(name="psum", bufs=1, space="PSUM"))
        ptp = ctx.enter_context(tc.tile_pool(name="ptp", bufs=5))
        nrm = ctx.enter_context(tc.tile_pool(name="nrm", bufs=4))
        osb_p = ctx.enter_context(tc.tile_pool(name="osbp", bufs=4))

        F8 = mybir.dt.float8e4
        LN32 = 3.4657359027997265  # exp shift so attention weights fit e4m3

        xts = sb.tile([P, KT_TILES, T], DT, name="xts")

        def xsl(kti, sl):
            return xts[:, kti, sl]

        # Q^T/K^T with the two heads stacked along partitions: rows 0:64 =
        # head0, 64:128 = head1 -> concurrent row-group score matmuls
        qt2 = sb.tile([P, T], DT, name="qt2")
        kt2 = sb.tile([P, T], DT, name="kt2")
        # V_aug in fp8, paired for DoubleRow: [tok, kb-pair, parity, head,
        # 64 V cols + 64 ones cols]
        va = sb.tile([P, NTB // 2, 2, 2, P], F8)
        wq_s = sb.tile([P, KT_TILES, DC], DT)
        wk_s = sb.tile([P, KT_TILES, DC], DT)
        wv_s = sb.tile([P, KT_TILES, DC], DT)
        wo_s = sb.tile([P, KT_TILES, D], DT)
        ctxt = [sb.tile([P, KT_TILES, CH], DT, name=f"ctxt{b}") for b in range(B)]
        ones = sb.tile([1, P], DT)
        bq_s = sb.tile([DC, 1], f32)
        bk_s = sb.tile([DC, 1], f32)
        bv_s = sb.tile([1, DC], DT)
        bo_s = sb.tile([1, D], DT)
        warm = sb.tile([1, 32], f32)
        nln32 = sb.tile([P, 1], f32, name="nln32")

        nc.vector.memset(nln32[:], -LN32)
        nc.vector.memset(ones[:], 1.0)
        # big fp8 memset (ones columns [.., 64:128] survive) on gpsimd: on
        # DVE it takes ~7us and delays the first projection relu
        nc.gpsimd.memset(va[:], 1.0)
        nc.vector.memset(warm[:], 0.0)
        nc.scalar.activation(warm[:], warm[:], AF.Exp, scale=1.0)

        # ---- warm-up collective FIRST: its ~20us ncfw first-call cost must
        # finish before the first real collective (~60us), and its input DMA
        # must not queue behind the x loads
        wcc_in = dram.tile([NCORES, 16, 16], DT)
        wcc_out = dram.tile([NCORES, 16, 16], DT)
        wcc_sb = sb.tile([16, NCORES * 16], DT)
        nc.vector.memset(wcc_sb[:], 0.0)
        nc.sync.dma_start(out=wcc_in[:].rearrange("j p c -> p j c"),
                          in_=wcc_sb[:].rearrange("p (j c) -> p j c", j=NCORES))
        nc.gpsimd.collective_compute(
            "AllToAll", mybir.AluOpType.bypass,
            replica_groups=[list(range(NCORES))],
            ins=[wcc_in.opt()], outs=[wcc_out.opt()],
        )

        if with_bias_qk:
            nc.sync.dma_start(out=bq_s[:], in_=bqd.ap())
            nc.sync.dma_start(out=bk_s[:], in_=bkd.ap())
        if with_bias_v:
            nc.sync.dma_start(out=bv_s[:], in_=bvd.ap())
        if with_bias_o:
            nc.sync.dma_start(out=bo_s[:], in_=bod.ap())

        # input DMAs, all on the sync queue (multi-queue splits of the same
        # tile serialize through write-tracking semaphores and regress):
        # wq/wk + chunk 0 first, then wv, remaining chunks.  wo on gpsimd.
        xT3 = xT.ap().rearrange("(k p) t -> k p t", p=P)
        nc.sync.dma_start(out=wq_s[:], in_=wq.ap().rearrange("(k p) c -> p k c", p=P))
        for kti in range(KT_TILES):
            nc.sync.dma_start(out=xts[:, kti, ts(0, 512)],
                              in_=xT3[kti][:, ts(0, 512)])
        nc.sync.dma_start(out=wk_s[:], in_=wk.ap().rearrange("(k p) c -> p k c", p=P))
        nc.sync.dma_start(out=wv_s[:], in_=wv.ap().rearrange("(k p) c -> p k c", p=P))
        for qcg in range(1, T // 512):
            for kti in range(KT_TILES):
                nc.sync.dma_start(out=xts[:, kti, ts(qcg, 512)],
                                  in_=xT3[kti][:, ts(qcg, 512)])
        wo3 = wo.ap().rearrange("(k p) e -> k p e", p=P)
        for kti in range(KT_TILES):
            nc.gpsimd.dma_start(out=wo_s[:, kti], in_=wo3[kti])

        # junk matmuls: keep PE busy during the input-DMA wait so HAM
        # un-throttles (2.4GHz) before the real projections start.  Few and
        # wide — each matmul drags a ~107ns LDWEIGHTS.
        wps = psum.tile([1, P], f32, tag="proj", bufs=2, name="warmmm")
        for _ in range(24):
            nc.tensor.matmul(wps[:], ones[:, 0:1], wcc_sb[0:1, 0:P],
                             start=True, stop=True)

        # ---- AllToAll buffers.  b0 + b1 first half: [8, 128, 128] halves
        # (token cols (qc%2)*64); b1 qc2/qc3: [8, 128, 64] singles
        a2a_h_in = [[dram.tile([NCORES, P, 2 * SLIV], DT, name=f"ahi{b}_{hh}")
                     for hh in range(2)] for b in range(B)]
        a2a_h_out = [[dram.tile([NCORES, P, 2 * SLIV], DT, name=f"aho{b}_{hh}")
                      for hh in range(2)] for b in range(B)]
        a2a_s_in = [dram.tile([NCORES, P, SLIV], DT, name=f"asi{qc}")
                    for qc in (2, 3)]
        a2a_s_out = [dram.tile([NCORES, P, SLIV], DT, name=f"aso{qc}")
                     for qc in (2, 3)]

        def trigger(in_t, out_t):
            nc.gpsimd.collective_compute(
                "AllToAll", mybir.AluOpType.bypass,
                replica_groups=[list(range(NCORES))],
                ins=[in_t.opt()], outs=[out_t.opt()],
            )

        # scatter destination for a given (batch, qc): (tile, column offset)
        def sc_dst(b, qc):
            if b == 0 or qc < 2:
                return a2a_h_in[b][qc // 2], (qc % 2) * SLIV
            return a2a_s_in[qc - 2], 0

        # collectives triggered right after a chunk's scatter lands
        ship = [{1: (a2a_h_in[0][0], a2a_h_out[0][0]),
                 3: (a2a_h_in[0][1], a2a_h_out[0][1])},
                {1: (a2a_h_in[1][0], a2a_h_out[1][0]),
                 2: (a2a_s_in[0], a2a_s_out[0]),
                 3: (a2a_s_in[1], a2a_s_out[1])}]

        def gather_half(b, hh, eng=None):
            e = eng or nc.gpsimd
            for i in range(NCORES):
                e.dma_start(out=ctxt[b][:, i, ts(hh, 2 * SLIV)],
                            in_=a2a_h_out[b][hh][i])

        def gather_qc2(eng=None):
            e = eng or nc.gpsimd
            for i in range(NCORES):
                e.dma_start(out=ctxt[1][:, i, ts(2, SLIV)],
                            in_=a2a_s_out[0][i])

        # ---- building blocks ----
        def _proj_qk_part(cell, qcg, w_s, b_s, dst2, wb, tag, lo, hi):
            if lo == 0:
                cell["ps"] = psum.tile([P, 512], f32, tag=tag, bufs=2,
                                       name=f"pqk{qcg}")
            ps = cell["ps"]
            for kti in range(lo, hi):
                nc.tensor.matmul(ps[:], w_s[:, kti], xsl(kti, ts(qcg, 512)),
                                 start=(kti == 0), stop=(kti == KT_TILES - 1))
            if hi < KT_TILES:
                return
            if wb:
                for h in range(2):
                    nc.scalar.activation(dst2[h * HD:(h + 1) * HD, ts(qcg, 512)],
                                         ps[h * HD:(h + 1) * HD, :],
                                         AF.Relu, bias=b_s[h * HD:(h + 1) * HD, :])
            else:
                nc.vector.tensor_scalar_max(dst2[:, ts(qcg, 512)], ps[:], 0.0)

        def proj_qk(qcg, w_s, b_s, dst2, wb, tag):
            _proj_qk_part({}, qcg, w_s, b_s, dst2, wb, tag, 0, KT_TILES)

        # ~1.3us filler units: PE bursts longer than ~2us starve ACT (the
        # PE queue is strict FIFO, so scores queue behind filler matmuls)
        def proj_qk_units(qcg, w_s, b_s, dst2, wb, tag="proj"):
            cell = {}
            return [
                lambda: _proj_qk_part(cell, qcg, w_s, b_s, dst2, wb, tag, 0, 4),
                lambda: _proj_qk_part(cell, qcg, w_s, b_s, dst2, wb, tag,
                                      4, KT_TILES),
            ]

        def proj_v(tb, tag):
            vps = psum.tile([P, DC], f32, tag=tag, bufs=2, name=f"pv{tb}")
            if with_bias_v:
                nc.tensor.matmul(vps[:], ones[:], bv_s[:], start=True, stop=False)
            for kti in range(KT_TILES):
                nc.tensor.matmul(vps[:], xsl(kti, ts(tb, P)), wv_s[:, kti],
                                 start=(kti == 0 and not with_bias_v),
                                 stop=(kti == KT_TILES - 1))
            for h in range(2):
                nc.vector.tensor_scalar_max(va[:, tb // 2, tb % 2, h, 0:HD],
                                            vps[:, h * HD:(h + 1) * HD], 0.0)

        def _outproj_part(cell, b, tb, ec, tag, lo, hi):
            if lo == 0:
                cell[ec] = psum.tile([P, 512], f32, tag=tag, bufs=2,
                                     name=f"po{b}_{tb}_{ec}")
                if with_bias_o:
                    nc.tensor.matmul(cell[ec][:], ones[:], bo_s[:, ts(ec, 512)],
                                     start=True, stop=False)
            ps = cell[ec]
            for kti in range(lo, hi):
                nc.tensor.matmul(ps[:], ctxt[b][:, kti, ts(tb, P)],
                                 wo_s[:, kti, ts(ec, 512)],
                                 start=(kti == 0 and not with_bias_o),
                                 stop=(kti == KT_TILES - 1))
            if hi < KT_TILES:
                return
            osb = osb_p.tile([P, 512], f32, tag="osb")
            nc.vector.tensor_scalar_max(osb[:], ps[:], 0.0)
            nc.sync.dma_start(out=out.ap()[ds(b * CH + tb * P, P), ts(ec, 512)],
                              in_=osb[:])

        def outproj_block(b, tb, tag="proj"):
            cell = {}
            for ec in range(D // 512):
                _outproj_part(cell, b, tb, ec, tag, 0, KT_TILES)

        def outproj_units(b, tb, tag="proj"):
            cell = {}
            units = []
            for ec in range(D // 512):
                units.append(lambda e=ec: _outproj_part(cell, b, tb, e, tag, 0, 4))
                units.append(lambda e=ec: _outproj_part(cell, b, tb, e, tag,
                                                        4, KT_TILES))
            return units

        # ---- attention for one batch; fillers fire at fractional positions
        def attention(b, fillers, positions, last=False):
            order = sorted(range(len(fillers)), key=lambda i: positions[i])
            fi = 0
            n_iter = SB_Q * KB
            it = 0
            for qc in range(SB_Q):
                qsl = ds(b * S + qc * 512, 512)
                cps = [psum.tile([P, 512], f32, tag="ctx", bufs=2,
                                 name=f"cps{b}_{qc}_{h}") for h in range(2)]
                pt = None
                for kb in range(KB):
                    ksl = ds(b * S + kb * P, P)
                    sps = psum.tile([P, 2, 512], f32, tag="sc", bufs=2)
                    for h in range(2):
                        # heads at PE row groups 0 / 64: concurrent streams
                        nc.tensor.matmul(sps[:, h],
                                         kt2[ds(h * HD, HD), ksl],
                                         qt2[ds(h * HD, HD), qsl],
                                         start=True, stop=True)
                    if kb % 2 == 0:
                        pt = ptp.tile([P, 2, 2, 512], F8, tag="p")
                    nc.scalar.activation(pt[:, kb % 2], sps[:], AF.Exp,
                                         scale=0.125, bias=nln32[:])
                    # fire fillers before the ctx emission so a filler at
                    # position (kb+eps) still precedes iteration kb+1's
                    # consumers in program order (V blocks ride as fillers)
                    it += 1
                    while fi < len(order) and positions[order[fi]] * n_iter < it:
                        fillers[order[fi]]()
                        fi += 1
                    if kb % 2 == 1:
                        pr = (b * KB + kb) // 2
                        for h in range(2):
                            nc.tensor.matmul(
                                cps[h][:], va[:, pr, :, h, :], pt[:, :, h, :],
                                start=(kb == 1), stop=(kb == KB - 1),
                                perf_mode=mybir.MatmulPerfMode.DoubleRow)
                # normalize; PSUM-releasing copies first.  Last chunk reads
                # straight from PSUM (no successor needs the slot).
                if last and qc == SB_Q - 1:
                    srcs = cps
                else:
                    cfull = [nrm.tile([P, 512], f32, tag=f"cf{h}", name=f"cf{h}")
                             for h in range(2)]
                    for h in range(2):
                        nc.vector.tensor_copy(cfull[h][:], cps[h][:])
                    srcs = cfull
                dst_t, coff = sc_dst(b, qc)
                for h in range(2):
                    recb = nrm.tile([HD, 512], f32, tag="recb")
                    if USE_FAST_RECIP:
                        # custom-DVE op needs aligned partitions: shift the
                        # denominator rows to base 0 with a regular copy
                        # (ACT does it on the final chunk -- ACT is idle and
                        # the DVE chain to the last collective shortens)
                        den0 = nrm.tile([HD, 512], f32, tag="den0")
                        if last and qc == SB_Q - 1:
                            nc.scalar.copy(den0[:], srcs[h][HD:P, :])
                        else:
                            nc.vector.tensor_copy(den0[:], srcs[h][HD:P, :])
                        nc.vector.reciprocal_approx_fast(recb[:], den0[:])
                    else:
                        nc.vector.reciprocal(recb[:], srcs[h][HD:P, :])
                    csb = nrm.tile([HD, 512], DT, tag="csb")
                    nc.vector.tensor_tensor(csb[:], srcs[h][0:HD, :], recb[:],
                                            mybir.AluOpType.mult)
                    nc.sync.dma_start(
                        out=dst_t[:, h * HD:(h + 1) * HD, ds(coff, SLIV)]
                            .rearrange("j p c -> p j c"),
                        in_=csb[:].rearrange("p (j c) -> p j c", j=NCORES))
                if qc in ship[b]:
                    trigger(*ship[b][qc])
            for i in order[fi:]:
                fillers[i]()

        # ================= schedule =================
        # minimal batch-0 prologue: q(qc0), k(qcg0), v(tb0..3)
        proj_qk(0, wq_s, bq_s, qt2, with_bias_qk, tag="ctx")
        proj_qk(0, wk_s, bk_s, kt2, with_bias_qk, tag="ctx")
        for tb in range(0, 4):
            proj_v(tb, tag="ctx")

        def add_units(fillers, pos, units, p0, dp):
            for i, u in enumerate(units):
                fillers.append(u)
                pos.append(p0 + i * dp)

        # batch-0 attention fillers
        fillers, pos = [], []
        for qcg in range(1, SB_Q):        # k chunk qcg needed at iter 4*qcg
            add_units(fillers, pos,
                      proj_qk_units(qcg, wk_s, bk_s, kt2, with_bias_qk),
                      (4 * qcg - 2.8) / 64, 1.2 / 64)
        for tb in range(4, 16):           # v(tb) consumed by ctx at iter tb|1
            fillers.append(lambda t=tb: proj_v(t, "proj"))
            pos.append((tb - 2.2) / 64)
        for qcj in range(1, SB_Q):        # q(qcj) needed at iter 16*qcj
            add_units(fillers, pos,
                      proj_qk_units(qcj, wq_s, bq_s, qt2, with_bias_qk),
                      (16 * qcj - 5) / 64, 1.5 / 64)
        # batch-1 earliest needs: k(qcg4) + q(qc0) + v(tb16..19)
        add_units(fillers, pos,
                  proj_qk_units(SB_Q, wk_s, bk_s, kt2, with_bias_qk),
                  0.40, 2.0 / 64)
        add_units(fillers, pos,
                  proj_qk_units(SB_Q, wq_s, bq_s, qt2, with_bias_qk),
                  0.70, 2.0 / 64)
        for i, tb in enumerate(range(16, 20)):
            fillers.append(lambda t=tb: proj_v(t, "proj"))
            pos.append(0.44 + 0.06 * i)
        # batch-0 H0 collective completes ~iter 42 (more under peer skew);
        # gather rides on gpsimd (never blocks PE), block A waits further
        fillers.append(lambda: gather_half(0, 0))
        pos.append(48 / 64)
        add_units(fillers, pos, outproj_units(0, 0), 53 / 64, 2.2 / 64)
        attention(0, fillers, pos)

        # batch-1 attention fillers
        fillers, pos = [], []
        for j, qcg in enumerate(range(SB_Q + 1, 2 * SB_Q)):  # k(qcg5..7)
            add_units(fillers, pos,
                      proj_qk_units(qcg, wk_s, bk_s, kt2, with_bias_qk),
                      (4 * (j + 1) - 2.8) / 64, 1.2 / 64)
        for tb in range(20, 32):
            fillers.append(lambda t=tb: proj_v(t, "proj"))
            pos.append((tb - 16 - 2.2) / 64)
        for qcj in range(1, SB_Q):
            add_units(fillers, pos,
                      proj_qk_units(SB_Q + qcj, wq_s, bq_s, qt2, with_bias_qk),
                      (16 * qcj - 5) / 64, 1.5 / 64)
        # batch-0 H1 collective completes early in this batch
        fillers.append(lambda: gather_half(0, 1))
        pos.append(0.14)
        add_units(fillers, pos, outproj_units(0, 1), 0.30, 2.2 / 64)
        # batch-1 H0 collective (posted iter 32) completes ~iter 42
        fillers.append(lambda: gather_half(1, 0))
        pos.append(46 / 64)
        # batch-1 qc2 collective (posted iter 48) completes ~iter 58
        fillers.append(lambda: gather_qc2())
        pos.append(59.5 / 64)
        attention(1, fillers, pos, last=True)

        # tail: block A (data long since gathered) covers the qc3 collective;
        # then only qc3's 64KB exchange + block B remain
        outproj_block(1, 0, tag="proj")
        # keepalive matmuls bridge the collective wait so HAM stays at full
        # clock for block B (results unused; "sc" slots are free post-exp)
        wps2 = psum.tile([1, P], f32, tag="sc", bufs=2, name="tailwm")
        for _ in range(24):
            nc.tensor.matmul(wps2[:], ones[:, 0:1], wcc_sb[0:1, 0:P],
                             start=True, stop=True)
        for i in range(NCORES):
            e = (nc.sync, nc.scalar)[i % 2]
            e.dma_start(out=ctxt[1][:, i, ts(SB_Q - 1, SLIV)],
                        in_=a2a_s_out[1][i])
        outproj_block(1, 1, tag="ctx")

    nc.compile()
    return nc


def _get(with_bias_v, with_bias_o, with_bias_qk):
    key = (with_bias_v, with_bias_o, with_bias_qk)
    if key not in _CACHE:
        _CACHE[key] = _build(*key)
    return _CACHE[key]


def kernel(x, Wq, bq, Wk, bk, Wv, bv, Wo, bo):
    global LAST_RESULTS
    from concourse.bass_utils import run_bass_kernel_spmd

    x = np.asarray(x, dtype=np.float32)
    Wq, Wk, Wv, Wo = (np.asarray(w, dtype=np.float32) for w in (Wq, Wk, Wv, Wo))
    bq, bk, bv, bo = (np.asarray(v, dtype=np.float32) for v in (bq, bk, bv, bo))

    wb_qk = bool(np.any(bq) or np.any(bk))
    wb_v = bool(np.any(bv))
    wb_o = bool(np.any(bo))
    nc = _get(wb_v, wb_o, wb_qk)

    xT = np.ascontiguousarray(x.reshape(T, D).astype(_bf).T)
    Wq16 = Wq.astype(_bf)
    Wk16 = Wk.astype(_bf)
    Wv16 = Wv.astype(_bf)
    Wo16 = np.ascontiguousarray(Wo.astype(_bf))
    bv16 = bv.astype(_bf)
    bo16 = np.ascontiguousarray(bo.astype(_bf).reshape(1, D))

    in_maps = []
    for c in range(NCORES):
        cs = slice(c * DC, (c + 1) * DC)
        in_maps.append({
            "xT": xT,
            "wq": np.ascontiguousarray(Wq16[:, cs]),
            "wk": np.ascontiguousarray(Wk16[:, cs]),
            "wv": np.ascontiguousarray(Wv16[:, cs]),
            "wo": Wo16,
            "bqv": np.ascontiguousarray(bq[cs].reshape(DC, 1)),
            "bkv": np.ascontiguousarray(bk[cs].reshape(DC, 1)),
            "bvv": np.ascontiguousarray(bv16[cs].reshape(1, DC)),
            "bov": bo16,
        })

    kw = {}
    if PROFILE:
        kw = dict(trace=True, trace_cores=PROFILE_CORES)
    res = run_bass_kernel_spmd(nc, in_maps, core_ids=list(range(NCORES)), **kw)
    LAST_RESULTS = res

    # core j, batch b, row (qc*64 + t) -> global token b*S + qc*512 + j*64 + t
    full = np.empty((T, D), np.float32)
    for j in range(NCORES):
        o = res.results[j]["out"]
        for b in range(B):
            blk = o[b * CH:(b + 1) * CH].reshape(SB_Q, SLIV, D)
            for qc in range(SB_Q):
                full[b * S + qc * 512 + j * SLIV:
                     b * S + qc * 512 + (j + 1) * SLIV] = blk[qc]
    return np.ascontiguousarray(full.reshape(B, S, D))


# revision 37
# speedup vs baseline: 1.0007x; 1.0007x over previous
"""Multi-head attention forward on 8 Trainium2 NeuronCores.

Problem (all shapes hardcoded): B=2, S=2048, D=1024, H=16, HD=64
    q = relu(x @ Wq + bq); k = relu(x @ Wk + bk); v = relu(x @ Wv + bv)
    attn = softmax(q k^T / sqrt(HD)) per (batch, head)
    out = relu((attn @ v) @ Wo + bo)

Sharding: head-parallel for QKV+attention (2 heads per core, both batches).
Token ownership for the output projection is sliver-interleaved: core j owns
tokens [qc*512 + j*64, qc*512 + (j+1)*64) for every query chunk qc.  Context
ships through AllToAlls sized so the CC stream never backlogs: batch-0 and
batch-1's first half ship per half-batch ([8,128,128]); batch-1's qc2/qc3
ship per-chunk so only a 64KB exchange sits in the tail.

Device schedule (per core):
  - Q^T/K^T as stacked 2-head [128, T] tiles; the two per-head score matmuls
    (K_c=64) land on different PE row groups and stream concurrently.
  - scores transposed S^T[k, q]; exp on ACT from PSUM with scale=1/8 and
    bias=-ln(32) (keeps weights in e4m3 range; cancels in the softmax).
    exp output is fp8 packed in kb pairs; attention*V runs as fp8 DoubleRow
    matmuls contracting 256 keys per instruction.  V_aug rows 64:128
    replicate the softmax denominator (ones columns).
  - normalize: regular-copy partition shift + reciprocal_approx_fast + mult.
  - all projections/gathers/output-projection blocks ride as small (~1.3us)
    filler units at deadline-driven positions inside the attention loops --
    PE executes its queue in order, so any burst longer than ~2us starves
    ACT (the bottleneck engine is whichever exceeds ~1.1us/iteration).
"""

import os
import sys

import numpy as np

for _p in ("/opt/trn_rl_repo",):
    if os.path.isdir(_p) and _p not in sys.path:
        sys.path.append(_p)

import ml_dtypes

B, S, D, H = 2, 2048, 1024, 16
HD = D // H          # 64
NCORES = 8
T = B * S            # 4096 flattened tokens
DC = D // NCORES     # 128 head-dim columns per core (2 heads)
P = 128
KT_TILES = D // P    # 8 contraction tiles over d_model
SB_Q = S // 512      # 4 query chunks per batch
KB = S // P          # 16 key blocks per batch
NTB = T // P         # 32 token blocks
SLIV = 512 // NCORES  # 64-token sliver per (qc, dest core)
CH = SB_Q * SLIV     # 256 tokens per core per batch

_bf = ml_dtypes.bfloat16

PROFILE = False
PROFILE_CORES = [0]
LAST_RESULTS = None
USE_FAST_RECIP = True

_CACHE = {}


def _build(with_bias_v, with_bias_o, with_bias_qk):
    import concourse.mybir as mybir
    import concourse.tile as tile
    from concourse import bacc
    from concourse.bass import ds, ts
    from contextlib import ExitStack

    f32 = mybir.dt.float32
    bf16 = mybir.dt.bfloat16
    DT = bf16
    AF = mybir.ActivationFunctionType

    nc = bacc.Bacc("TRN2", target_bir_lowering=False, debug=False,
                   num_devices=NCORES)

    xT = nc.dram_tensor("xT", [D, T], DT, kind="ExternalInput")
    wq = nc.dram_tensor("wq", [D, DC], DT, kind="ExternalInput")
    wk = nc.dram_tensor("wk", [D, DC], DT, kind="ExternalInput")
    wv = nc.dram_tensor("wv", [D, DC], DT, kind="ExternalInput")
    wo = nc.dram_tensor("wo", [D, D], DT, kind="ExternalInput")
    bqd = nc.dram_tensor("bqv", [DC, 1], f32, kind="ExternalInput")
    bkd = nc.dram_tensor("bkv", [DC, 1], f32, kind="ExternalInput")
    bvd = nc.dram_tensor("bvv", [1, DC], DT, kind="ExternalInput")
    bod = nc.dram_tensor("bov", [1, D], DT, kind="ExternalInput")
    out = nc.dram_tensor("out", [B * CH, D], f32, kind="ExternalOutput")

    with tile.TileContext(nc) as tc, ExitStack() as ctx:
        sb = ctx.enter_context(tc.tile_pool(name="persist", bufs=1))
        dram = ctx.enter_context(tc.tile_pool(name="dram", bufs=1, space="DRAM"))
        psum = ctx.enter_context(tc.tile_pool(name="psum", bufs=1, space="PSUM"))
        ptp = ctx.enter_context(tc.tile_pool(name="ptp", bufs=5))
        nrm = ctx.enter_context(tc.tile_pool(name="nrm", bufs=4))
        osb_p = ctx.enter_context(tc.tile_pool(name="osbp", bufs=4))

        F8 = mybir.dt.float8e4
        LN32 = 3.4657359027997265  # exp shift so attention weights fit e4m3

        xts = sb.tile([P, KT_TILES, T], DT, name="xts")

        def xsl(kti, sl):
            return xts[:, kti, sl]

        # Q^T/K^T with the two heads stacked along partitions: rows 0:64 =
        # head0, 64:128 = head1 -> concurrent row-group score matmuls
        qt2 = sb.tile([P, T], DT, name="qt2")
        kt2 = sb.tile([P, T], DT, name="kt2")
        # V_aug in fp8, paired for DoubleRow: [tok, kb-pair, parity, head,
        # 64 V cols + 64 ones cols]
        va = sb.tile([P, NTB // 2, 2, 2, P], F8)
        wq_s = sb.tile([P, KT_TILES, DC], DT)
        wk_s = sb.tile([P, KT_TILES, DC], DT)
        wv_s = sb.tile([P, KT_TILES, DC], DT)
        wo_s = sb.tile([P, KT_TILES, D], DT)
        ctxt = [sb.tile([P, KT_TILES, CH], DT, name=f"ctxt{b}") for b in range(B)]
        ones = sb.tile([1, P], DT)
        bq_s = sb.tile([DC, 1], f32)
        bk_s = sb.tile([DC, 1], f32)
        bv_s = sb.tile([1, DC], DT)
        bo_s = sb.tile([1, D], DT)
        warm = sb.tile([1, 32], f32)
        nln32 = sb.tile([P, 1], f32, name="nln32")

        nc.vector.memset(nln32[:], -LN32)
        nc.vector.memset(ones[:], 1.0)
        # big fp8 memset (ones columns [.., 64:128] survive) on gpsimd: on
        # DVE it takes ~7us and delays the first projection relu
        nc.gpsimd.memset(va[:], 1.0)
        nc.vector.memset(warm[:], 0.0)
        nc.scalar.activation(warm[:], warm[:], AF.Exp, scale=1.0)

        # ---- warm-up collective FIRST: its ~20us ncfw first-call cost must
        # finish before the first real collective (~60us), and its input DMA
        # must not queue behind the x loads
        wcc_in = dram.tile([NCORES, 16, 16], DT)
        wcc_out = dram.tile([NCORES, 16, 16], DT)
        wcc_sb = sb.tile([16, NCORES * 16], DT)
        nc.vector.memset(wcc_sb[:], 0.0)
        nc.sync.dma_start(out=wcc_in[:].rearrange("j p c -> p j c"),
                          in_=wcc_sb[:].rearrange("p (j c) -> p j c", j=NCORES))
        nc.gpsimd.collective_compute(
            "AllToAll", mybir.AluOpType.bypass,
            replica_groups=[list(range(NCORES))],
            ins=[wcc_in.opt()], outs=[wcc_out.opt()],
        )

        if with_bias_qk:
            nc.sync.dma_start(out=bq_s[:], in_=bqd.ap())
            nc.sync.dma_start(out=bk_s[:], in_=bkd.ap())
        if with_bias_v:
            nc.sync.dma_start(out=bv_s[:], in_=bvd.ap())
        if with_bias_o:
            nc.sync.dma_start(out=bo_s[:], in_=bod.ap())

        # input DMAs, all on the sync queue (multi-queue splits of the same
        # tile serialize through write-tracking semaphores and regress):
        # wq/wk + chunk 0 first, then wv, remaining chunks.  wo on gpsimd.
        xT3 = xT.ap().rearrange("(k p) t -> k p t", p=P)
        nc.sync.dma_start(out=wq_s[:], in_=wq.ap().rearrange("(k p) c -> p k c", p=P))
        nc.sync.dma_start(out=wk_s[:], in_=wk.ap().rearrange("(k p) c -> p k c", p=P))
        for kti in range(KT_TILES):
            nc.sync.dma_start(out=xts[:, kti, ts(0, 512)],
                              in_=xT3[kti][:, ts(0, 512)])
        nc.sync.dma_start(out=wv_s[:], in_=wv.ap().rearrange("(k p) c -> p k c", p=P))
        for qcg in range(1, T // 512):
            for kti in range(KT_TILES):
                nc.sync.dma_start(out=xts[:, kti, ts(qcg, 512)],
                                  in_=xT3[kti][:, ts(qcg, 512)])
        wo3 = wo.ap().rearrange("(k p) e -> k p e", p=P)
        for kti in range(KT_TILES):
            nc.gpsimd.dma_start(out=wo_s[:, kti], in_=wo3[kti])

        # junk matmuls: keep PE busy during the input-DMA wait so HAM
        # un-throttles (2.4GHz) before the real projections start.  Few and
        # wide — each matmul drags a ~107ns LDWEIGHTS.
        wps = psum.tile([1, P], f32, tag="proj", bufs=2, name="warmmm")
        for _ in range(14):
            nc.tensor.matmul(wps[:], ones[:, 0:1], wcc_sb[0:1, 0:P],
                             start=True, stop=True)

        # ---- AllToAll buffers.  b0 + b1 first half: [8, 128, 128] halves
        # (token cols (qc%2)*64); b1 qc2/qc3: [8, 128, 64] singles
        a2a_h_in = [[dram.tile([NCORES, P, 2 * SLIV], DT, name=f"ahi{b}_{hh}")
                     for hh in range(2)] for b in range(B)]
        a2a_h_out = [[dram.tile([NCORES, P, 2 * SLIV], DT, name=f"aho{b}_{hh}")
                      for hh in range(2)] for b in range(B)]
        a2a_s_in = [dram.tile([NCORES, P, SLIV], DT, name=f"asi{qc}")
                    for qc in (2, 3)]
        a2a_s_out = [dram.tile([NCORES, P, SLIV], DT, name=f"aso{qc}")
                     for qc in (2, 3)]

        def trigger(in_t, out_t):
            nc.gpsimd.collective_compute(
                "AllToAll", mybir.AluOpType.bypass,
                replica_groups=[list(range(NCORES))],
                ins=[in_t.opt()], outs=[out_t.opt()],
            )

        # scatter destination for a given (batch, qc): (tile, column offset)
        def sc_dst(b, qc):
            if b == 0 or qc < 2:
                return a2a_h_in[b][qc // 2], (qc % 2) * SLIV
            return a2a_s_in[qc - 2], 0

        # collectives triggered right after a chunk's scatter lands
        ship = [{1: (a2a_h_in[0][0], a2a_h_out[0][0]),
                 3: (a2a_h_in[0][1], a2a_h_out[0][1])},
                {1: (a2a_h_in[1][0], a2a_h_out[1][0]),
                 2: (a2a_s_in[0], a2a_s_out[0]),
                 3: (a2a_s_in[1], a2a_s_out[1])}]

        def gather_half(b, hh, eng=None):
            e = eng or nc.gpsimd
            for i in range(NCORES):
                e.dma_start(out=ctxt[b][:, i, ts(hh, 2 * SLIV)],
                            in_=a2a_h_out[b][hh][i])

        def gather_qc2(eng=None):
            e = eng or nc.gpsimd
            for i in range(NCORES):
                e.dma_start(out=ctxt[1][:, i, ts(2, SLIV)],
                            in_=a2a_s_out[0][i])

        # ---- building blocks ----
        def _proj_qk_part(cell, qcg, w_s, b_s, dst2, wb, tag, lo, hi):
            if lo == 0:
                cell["ps"] = psum.tile([P, 512], f32, tag=tag, bufs=2,
                                       name=f"pqk{qcg}")
            ps = cell["ps"]
            for kti in range(lo, hi):
                nc.tensor.matmul(ps[:], w_s[:, kti], xsl(kti, ts(qcg, 512)),
                                 start=(kti == 0), stop=(kti == KT_TILES - 1))
            if hi < KT_TILES:
                return
            if wb:
                for h in range(2):
                    nc.scalar.activation(dst2[h * HD:(h + 1) * HD, ts(qcg, 512)],
                                         ps[h * HD:(h + 1) * HD, :],
                                         AF.Relu, bias=b_s[h * HD:(h + 1) * HD, :])
            else:
                nc.vector.tensor_scalar_max(dst2[:, ts(qcg, 512)], ps[:], 0.0)

        def proj_qk(qcg, w_s, b_s, dst2, wb, tag):
            _proj_qk_part({}, qcg, w_s, b_s, dst2, wb, tag, 0, KT_TILES)

        # ~1.3us filler units: PE bursts longer than ~2us starve ACT (the
        # PE queue is strict FIFO, so scores queue behind filler matmuls)
        def proj_qk_units(qcg, w_s, b_s, dst2, wb, tag="proj"):
            cell = {}
            return [
                lambda: _proj_qk_part(cell, qcg, w_s, b_s, dst2, wb, tag, 0, 4),
                lambda: _proj_qk_part(cell, qcg, w_s, b_s, dst2, wb, tag,
                                      4, KT_TILES),
            ]

        def proj_v(tb, tag):
            vps = psum.tile([P, DC], f32, tag=tag, bufs=2, name=f"pv{tb}")
            if with_bias_v:
                nc.tensor.matmul(vps[:], ones[:], bv_s[:], start=True, stop=False)
            for kti in range(KT_TILES):
                nc.tensor.matmul(vps[:], xsl(kti, ts(tb, P)), wv_s[:, kti],
                                 start=(kti == 0 and not with_bias_v),
                                 stop=(kti == KT_TILES - 1))
            for h in range(2):
                nc.vector.tensor_scalar_max(va[:, tb // 2, tb % 2, h, 0:HD],
                                            vps[:, h * HD:(h + 1) * HD], 0.0)

        def _outproj_part(cell, b, tb, ec, tag, lo, hi):
            if lo == 0:
                cell[ec] = psum.tile([P, 512], f32, tag=tag, bufs=2,
                                     name=f"po{b}_{tb}_{ec}")
                if with_bias_o:
                    nc.tensor.matmul(cell[ec][:], ones[:], bo_s[:, ts(ec, 512)],
                                     start=True, stop=False)
            ps = cell[ec]
            for kti in range(lo, hi):
                nc.tensor.matmul(ps[:], ctxt[b][:, kti, ts(tb, P)],
                                 wo_s[:, kti, ts(ec, 512)],
                                 start=(kti == 0 and not with_bias_o),
                                 stop=(kti == KT_TILES - 1))
            if hi < KT_TILES:
                return
            osb = osb_p.tile([P, 512], f32, tag="osb")
            nc.vector.tensor_scalar_max(osb[:], ps[:], 0.0)
            nc.sync.dma_start(out=out.ap()[ds(b * CH + tb * P, P), ts(ec, 512)],
                              in_=osb[:])

        def outproj_block(b, tb, tag="proj"):
            cell = {}
            for ec in range(D // 512):
                _outproj_part(cell, b, tb, ec, tag, 0, KT_TILES)

        def outproj_units(b, tb, tag="proj"):
            cell = {}
            units = []
            for ec in range(D // 512):
                units.append(lambda e=ec: _outproj_part(cell, b, tb, e, tag, 0, 4))
                units.append(lambda e=ec: _outproj_part(cell, b, tb, e, tag,
                                                        4, KT_TILES))
            return units

        # ---- attention for one batch; fillers fire at fractional positions
        def attention(b, fillers, positions, last=False):
            order = sorted(range(len(fillers)), key=lambda i: positions[i])
            fi = 0
            n_iter = SB_Q * KB
            it = 0
            for qc in range(SB_Q):
                qsl = ds(b * S + qc * 512, 512)
                cps = [psum.tile([P, 512], f32, tag="ctx", bufs=2,
                                 name=f"cps{b}_{qc}_{h}") for h in range(2)]
                pt = None
                for kb in range(KB):
                    ksl = ds(b * S + kb * P, P)
                    sps = psum.tile([P, 2, 512], f32, tag="sc", bufs=2)
                    for h in range(2):
                        # heads at PE row groups 0 / 64: concurrent streams
                        nc.tensor.matmul(sps[:, h],
                                         kt2[ds(h * HD, HD), ksl],
                                         qt2[ds(h * HD, HD), qsl],
                                         start=True, stop=True)
                    if kb % 2 == 0:
                        pt = ptp.tile([P, 2, 2, 512], F8, tag="p")
                    nc.scalar.activation(pt[:, kb % 2], sps[:], AF.Exp,
                                         scale=0.125, bias=nln32[:])
                    # fire fillers before the ctx emission so a filler at
                    # position (kb+eps) still precedes iteration kb+1's
                    # consumers in program order (V blocks ride as fillers)
                    it += 1
                    while fi < len(order) and positions[order[fi]] * n_iter < it:
                        fillers[order[fi]]()
                        fi += 1
                    if kb % 2 == 1:
                        pr = (b * KB + kb) // 2
                        for h in range(2):
                            nc.tensor.matmul(
                                cps[h][:], va[:, pr, :, h, :], pt[:, :, h, :],
                                start=(kb == 1), stop=(kb == KB - 1),
                                perf_mode=mybir.MatmulPerfMode.DoubleRow)
                # normalize; PSUM-releasing copies first.  Last chunk reads
                # straight from PSUM (no successor needs the slot).
                if last and qc == SB_Q - 1:
                    srcs = cps
                else:
                    cfull = [nrm.tile([P, 512], f32, tag=f"cf{h}", name=f"cf{h}")
                             for h in range(2)]
                    for h in range(2):
                        nc.vector.tensor_copy(cfull[h][:], cps[h][:])
                    srcs = cfull
                dst_t, coff = sc_dst(b, qc)
                for h in range(2):
                    recb = nrm.tile([HD, 512], f32, tag="recb")
                    if USE_FAST_RECIP:
                        # custom-DVE op needs aligned partitions: shift the
                        # denominator rows to base 0 with a regular copy
                        # (ACT does it on the final chunk -- ACT is idle and
                        # the DVE chain to the last collective shortens)
                        den0 = nrm.tile([HD, 512], f32, tag="den0")
                        if last and qc == SB_Q - 1:
                            nc.scalar.copy(den0[:], srcs[h][HD:P, :])
                        else:
                            nc.vector.tensor_copy(den0[:], srcs[h][HD:P, :])
                        nc.vector.reciprocal_approx_fast(recb[:], den0[:])
                    else:
                        nc.vector.reciprocal(recb[:], srcs[h][HD:P, :])
                    csb = nrm.tile([HD, 512], DT, tag="csb")
                    nc.vector.tensor_tensor(csb[:], srcs[h][0:HD, :], recb[:],
                                            mybir.AluOpType.mult)
                    nc.sync.dma_start(
                        out=dst_t[:, h * HD:(h + 1) * HD, ds(coff, SLIV)]
                            .rearrange("j p c -> p j c"),
                        in_=csb[:].rearrange("p (j c) -> p j c", j=NCORES))
                if qc in ship[b]:
                    trigger(*ship[b][qc])
            for i in order[fi:]:
                fillers[i]()

        # ================= schedule =================
        # minimal batch-0 prologue: q(qc0), k(qcg0), v(tb0..3)
        proj_qk(0, wq_s, bq_s, qt2, with_bias_qk, tag="ctx")
        proj_qk(0, wk_s, bk_s, kt2, with_bias_qk, tag="ctx")
        for tb in range(0, 4):
            proj_v(tb, tag="ctx")

        def add_units(fillers, pos, units, p0, dp):
            for i, u in enumerate(units):
                fillers.append(u)
                pos.append(p0 + i * dp)

        # batch-0 attention fillers
        fillers, pos = [], []
        for qcg in range(1, SB_Q):        # k chunk qcg needed at iter 4*qcg
            add_units(fillers, pos,
                      proj_qk_units(qcg, wk_s, bk_s, kt2, with_bias_qk),
                      (4 * qcg - 2.8) / 64, 1.2 / 64)
        for tb in range(4, 16):           # v(tb) consumed by ctx at iter tb|1
            fillers.append(lambda t=tb: proj_v(t, "proj"))
            pos.append((tb - 2.2) / 64)
        for qcj in range(1, SB_Q):        # q(qcj) needed at iter 16*qcj
            add_units(fillers, pos,
                      proj_qk_units(qcj, wq_s, bq_s, qt2, with_bias_qk),
                      (16 * qcj - 5) / 64, 1.5 / 64)
        # batch-1 earliest needs: k(qcg4) + q(qc0) + v(tb16..19)
        add_units(fillers, pos,
                  proj_qk_units(SB_Q, wk_s, bk_s, kt2, with_bias_qk),
                  0.40, 2.0 / 64)
        add_units(fillers, pos,
                  proj_qk_units(SB_Q, wq_s, bq_s, qt2, with_bias_qk),
                  0.70, 2.0 / 64)
        for i, tb in enumerate(range(16, 20)):
            fillers.append(lambda t=tb: proj_v(t, "proj"))
            pos.append(0.44 + 0.06 * i)
        # batch-0 H0 collective completes ~iter 42 (more under peer skew);
        # gather rides on gpsimd (never blocks PE), block A waits further
        fillers.append(lambda: gather_half(0, 0))
        pos.append(48 / 64)
        add_units(fillers, pos, outproj_units(0, 0), 53 / 64, 2.2 / 64)
        attention(0, fillers, pos)

        # batch-1 attention fillers
        fillers, pos = [], []
        for j, qcg in enumerate(range(SB_Q + 1, 2 * SB_Q)):  # k(qcg5..7)
            add_units(fillers, pos,
                      proj_qk_units(qcg, wk_s, bk_s, kt2, with_bias_qk),
                      (4 * (j + 1) - 2.8) / 64, 1.2 / 64)
        for tb in range(20, 32):
            fillers.append(lambda t=tb: proj_v(t, "proj"))
            pos.append((tb - 16 - 2.2) / 64)
        for qcj in range(1, SB_Q):
            add_units(fillers, pos,
                      proj_qk_units(SB_Q + qcj, wq_s, bq_s, qt2, with_bias_qk),
                      (16 * qcj - 5) / 64, 1.5 / 64)
        # batch-0 H1 collective completes early in this batch
        fillers.append(lambda: gather_half(0, 1))
        pos.append(0.14)
        add_units(fillers, pos, outproj_units(0, 1), 0.30, 2.2 / 64)
        # batch-1 H0 collective (posted iter 32) completes ~iter 42
        fillers.append(lambda: gather_half(1, 0))
        pos.append(46 / 64)
        # batch-1 qc2 collective (posted iter 48) completes ~iter 58
        fillers.append(lambda: gather_qc2())
        pos.append(59.5 / 64)
        attention(1, fillers, pos, last=True)

        # tail: block A (data long since gathered) covers the qc3 collective;
        # then only qc3's 64KB exchange + block B remain
        outproj_block(1, 0, tag="proj")
        for i in range(NCORES):
            e = (nc.sync, nc.scalar)[i % 2]
            e.dma_start(out=ctxt[1][:, i, ts(SB_Q - 1, SLIV)],
                        in_=a2a_s_out[1][i])
        outproj_block(1, 1, tag="ctx")

    nc.compile()
    return nc


def _get(with_bias_v, with_bias_o, with_bias_qk):
    key = (with_bias_v, with_bias_o, with_bias_qk)
    if key not in _CACHE:
        _CACHE[key] = _build(*key)
    return _CACHE[key]


def kernel(x, Wq, bq, Wk, bk, Wv, bv, Wo, bo):
    global LAST_RESULTS
    from concourse.bass_utils import run_bass_kernel_spmd

    x = np.asarray(x, dtype=np.float32)
    Wq, Wk, Wv, Wo = (np.asarray(w, dtype=np.float32) for w in (Wq, Wk, Wv, Wo))
    bq, bk, bv, bo = (np.asarray(v, dtype=np.float32) for v in (bq, bk, bv, bo))

    wb_qk = bool(np.any(bq) or np.any(bk))
    wb_v = bool(np.any(bv))
    wb_o = bool(np.any(bo))
    nc = _get(wb_v, wb_o, wb_qk)

    xT = np.ascontiguousarray(x.reshape(T, D).astype(_bf).T)
    Wq16 = Wq.astype(_bf)
    Wk16 = Wk.astype(_bf)
    Wv16 = Wv.astype(_bf)
    Wo16 = np.ascontiguousarray(Wo.astype(_bf))
    bv16 = bv.astype(_bf)
    bo16 = np.ascontiguousarray(bo.astype(_bf).reshape(1, D))

    in_maps = []
    for c in range(NCORES):
        cs = slice(c * DC, (c + 1) * DC)
        in_maps.append({
            "xT": xT,
            "wq": np.ascontiguousarray(Wq16[:, cs]),
            "wk": np.ascontiguousarray(Wk16[:, cs]),
            "wv": np.ascontiguousarray(Wv16[:, cs]),
            "wo": Wo16,
            "bqv": np.ascontiguousarray(bq[cs].reshape(DC, 1)),
            "bkv": np.ascontiguousarray(bk[cs].reshape(DC, 1)),
            "bvv": np.ascontiguousarray(bv16[cs].reshape(1, DC)),
            "bov": bo16,
        })

    kw = {}
    if PROFILE:
        kw = dict(trace=True, trace_cores=PROFILE_CORES)
    res = run_bass_kernel_spmd(nc, in_maps, core_ids=list(range(NCORES)), **kw)
    LAST_RESULTS = res

    # core j, batch b, row (qc*64 + t) -> global token b*S + qc*512 + j*64 + t
    full = np.empty((T, D), np.float32)
    for j in range(NCORES):
        o = res.results[j]["out"]
        for b in range(B):
            blk = o[b * CH:(b + 1) * CH].reshape(SB_Q, SLIV, D)
            for qc in range(SB_Q):
                full[b * S + qc * 512 + j * SLIV:
                     b * S + qc * 512 + (j + 1) * SLIV] = blk[qc]
    return np.ascontiguousarray(full.reshape(B, S, D))
